# revision 1
# baseline (speedup 1.0000x reference)
"""BitLinear 2-bit quantized linear layer on 8 TRN2 NeuronCores.

Math: reference computes
    a      = clip(max|x| over last dim, EPS)
    out    = ((x/a) @ W_deq^T) * (a*scale) + bias,  W_deq = QUANT_LEVELS[codes]
The per-row absmax normalization cancels exactly (division by `a` then
multiplication by the same `a`), so out == (x @ W_deq^T) * scale + bias.
QUANT_LEVELS[c] = c - 1.5, so W_deq (and W_deq*scale for scale=1) is exactly
representable in bf16. We therefore run a plain bf16 matmul with fp32 PSUM
accumulation and a bias epilogue.

Sharding: data-parallel over the 8192 = 4*2048 (batch*seq) rows; each of the
8 cores computes a [1024, 4096] slice of the output with the full weight.
Host pre-transposes both operands so the device only does DMA + matmul:
  xT [K=4096, M=1024] bf16 per core, wT [K=4096, N=4096] bf16 replicated.
"""

import time

import numpy as np
import ml_dtypes

import concourse.mybir as mybir
from concourse import bacc
from concourse.tile import TileContext
from concourse.bass_utils import run_bass_kernel_spmd

N_CORES = 8
B, S, D_IN, D_OUT = 4, 2048, 4096, 4096
M_TOTAL = B * S              # 8192 rows
M = M_TOTAL // N_CORES       # 1024 rows per core
K = D_IN
N = D_OUT
P = 128                      # partitions
KI = K // P                  # 32 k-tiles
NF = 512                     # psum free dim (one PSUM bank of fp32)
NI = N // NF                 # 8 n-chunks
MI = M // P                  # 8 m-tiles

BF16 = mybir.dt.bfloat16
F32 = mybir.dt.float32


def build(m=M, k=K, n=N):
    ki, mi_n, ni_n = k // P, m // P, n // NF
    nc = bacc.Bacc()
    xT = nc.declare_dram_parameter("xT", [k, m], BF16, isOutput=False)
    wT = nc.declare_dram_parameter("wT", [k, n], BF16, isOutput=False)
    bias = nc.declare_dram_parameter("bias", [P, n], F32, isOutput=False)
    out = nc.declare_dram_parameter("out", [m, n], F32, isOutput=True)

    xT3 = xT[:].rearrange("(a p) m -> p a m", p=P)   # [128, ki, m]
    wT3 = wT[:].rearrange("(a p) n -> p a n", p=P)   # [128, ki, n]

    with TileContext(nc) as tc:
        with (
            tc.tile_pool(name="xpool", bufs=1) as xpool,
            tc.tile_pool(name="bpool", bufs=1) as bpool,
            tc.tile_pool(name="wpool", bufs=2) as wpool,
            tc.tile_pool(name="opool", bufs=6) as opool,
            tc.tile_pool(name="ppool", bufs=8, space="PSUM") as ppool,
        ):
            # x is resident for the whole kernel; the first W chunk and x are
            # loaded interleaved in ki-order pieces so ni=0 matmuls can start
            # after ~1.5 MiB instead of the full 12 MiB. x goes through the
            # ACT DGE ring and w through the SP ring so descriptor generation
            # for the two streams runs in parallel.
            xt = xpool.tile([P, ki, m], BF16, name="xt")
            wg = 8 if ki % 8 == 0 else 1
            kj = ki // wg
            wt0 = wpool.tile([P, ki, NF], BF16, name="wt")
            if wg > 1 and (ki - 4) % kj == 0:
                # smaller leading pieces so the first matmuls unblock sooner
                chunk_sizes = [1, 1, 2] + [kj] * ((ki - 4) // kj)
            else:
                chunk_sizes = [kj] * wg
            assert sum(chunk_sizes) == ki
            pos = 0
            for cs in chunk_sizes:
                sl = slice(pos, pos + cs)
                nc.scalar.dma_start(out=xt[:, sl, :], in_=xT3[:, sl, :])
                nc.sync.dma_start(out=wt0[:, sl, :], in_=wT3[:, sl, 0:NF])
                pos += cs
            bias_sb = bpool.tile([P, n], F32, name="bias_sb")
            nc.scalar.dma_start(out=bias_sb[:], in_=bias[:])

            # PE warmup: dummy matmuls on zeroed tiles keep the PE busy while
            # the first data chunks stream in, so the HAM clock-gate reaches
            # 2.4 GHz before the real accumulation starts (saves the ~10 us
            # cold-clock window). Results land in a psum bank that the real
            # ni=0 group overwrites (start=True resets the bank).
            warm_l = bpool.tile([P, P], BF16, name="warm_l")
            warm_r = bpool.tile([P, NF], BF16, name="warm_r")
            nc.vector.memset(warm_l[:], 0.0)
            nc.vector.memset(warm_r[:], 0.0)

            def epilogue(ps, mi, nsl):
                ot = opool.tile([P, NF], F32, name="ot")
                nc.vector.tensor_add(out=ot[:], in0=ps[:], in1=bias_sb[:, nsl])
                nc.sync.dma_start(out=out[mi * P:(mi + 1) * P, nsl], in_=ot[:])

            wt = wt0
            for ni in range(ni_n):
                nsl = slice(ni * NF, (ni + 1) * NF)
                wt_next = None
                if ni + 1 < ni_n:
                    wt_next = wpool.tile([P, ki, NF], BF16, name="wt")
                if ni == 0:
                    # ki-chunk-major over all 8 psum banks: accumulate into
                    # every mi's bank as each ki piece of x/w arrives, so PE
                    # rides right behind the startup DMA stream.
                    pss = [ppool.tile([P, NF], F32, name="ps") for _ in range(mi_n)]
                    for _ in range(12):
                        nc.tensor.matmul(
                            pss[mi_n - 1][:], lhsT=warm_l[:], rhs=warm_r[:],
                            start=True, stop=True,
                        )
                    cpos = 0
                    for gi, cs in enumerate(chunk_sizes):
                        for mi in range(mi_n):
                            for kk in range(cpos, cpos + cs):
                                nc.tensor.matmul(
                                    pss[mi][:],
                                    lhsT=xt[:, kk, mi * P:(mi + 1) * P],
                                    rhs=wt[:, kk, :],
                                    start=(kk == 0),
                                    stop=(kk == ki - 1),
                                )
                        cpos += cs
                        # delay the ni=1 weight prefetch until the startup
                        # stream is done so they don't race for HBM bandwidth
                        if gi == len(chunk_sizes) - 1 and wt_next is not None:
                            for g2 in range(wg):
                                sl = slice(g2 * kj, (g2 + 1) * kj)
                                nc.sync.dma_start(
                                    out=wt_next[:, sl, :],
                                    in_=wT3[:, sl, NF:2 * NF],
                                )
                    for mi in range(mi_n):
                        epilogue(pss[mi], mi, nsl)
                else:
                    for mi in range(mi_n):
                        last_group = ni == ni_n - 1 and mi == mi_n - 1
                        if last_group:
                            # split the final group into two pipelined halves
                            # so the kernel-tail drain only waits on a short
                            # half-width epilogue chain after the last matmul
                            hf = NF // 2
                            for half in range(2):
                                ps = ppool.tile([P, hf], F32, name="ps")
                                for kk in range(ki):
                                    nc.tensor.matmul(
                                        ps[:],
                                        lhsT=xt[:, kk, mi * P:(mi + 1) * P],
                                        rhs=wt[:, kk, half * hf:(half + 1) * hf],
                                        start=(kk == 0),
                                        stop=(kk == ki - 1),
                                    )
                                hsl = slice(ni * NF + half * hf,
                                            ni * NF + (half + 1) * hf)
                                ot = opool.tile([P, hf], F32, name="ot")
                                nc.vector.tensor_add(
                                    out=ot[:], in0=ps[:], in1=bias_sb[:, hsl])
                                nc.sync.dma_start(
                                    out=out[mi * P:(mi + 1) * P, hsl], in_=ot[:])
                            continue
                        ps = ppool.tile([P, NF], F32, name="ps")
                        for kk in range(ki):
                            nc.tensor.matmul(
                                ps[:],
                                lhsT=xt[:, kk, mi * P:(mi + 1) * P],
                                rhs=wt[:, kk, :],
                                start=(kk == 0),
                                stop=(kk == ki - 1),
                            )
                        epilogue(ps, mi, nsl)
                        # spread next-chunk weight DMA issue across the phase
                        if wt_next is not None and mi < wg:
                            sl = slice(mi * kj, (mi + 1) * kj)
                            nc.sync.dma_start(
                                out=wt_next[:, sl, :],
                                in_=wT3[:, sl, (ni + 1) * NF:(ni + 2) * NF],
                            )
                wt = wt_next
    nc.finalize()
    return nc


_NC = None


def _get_nc():
    global _NC
    if _NC is None:
        _NC = build()
    return _NC


def make_in_maps(x, weight_2bit, weight_scale, bias):
    x = np.asarray(x)
    codes = np.asarray(weight_2bit)
    ws = np.float32(np.asarray(weight_scale).reshape(-1)[0])
    b = np.asarray(bias).astype(np.float32)

    w_f = (codes.astype(np.float32) - np.float32(1.5)) * ws      # [N, K]
    wT = np.ascontiguousarray(w_f.T.astype(ml_dtypes.bfloat16))  # [K, N]
    bias_rep = np.ascontiguousarray(np.broadcast_to(b, (P, N)))

    x2 = x.reshape(M_TOTAL, K).astype(ml_dtypes.bfloat16)
    in_maps = []
    for c in range(N_CORES):
        xTc = np.ascontiguousarray(x2[c * M:(c + 1) * M].T)      # [K, M]
        in_maps.append({"xT": xTc, "wT": wT, "bias": bias_rep})
    return in_maps


def run(in_maps, trace=False, **kw):
    # The axon-tunneled devices occasionally fail a fresh process's first
    # execution with NRT_EXEC_UNIT_UNRECOVERABLE; an identical retry succeeds.
    last = None
    for attempt in range(4):
        try:
            return run_bass_kernel_spmd(
                _get_nc(), in_maps, list(range(N_CORES)), trace=trace, **kw
            )
        except Exception as e:
            last = e
            msg = str(e)
            if "UNAVAILABLE" in msg or "unrecoverable" in msg.lower():
                # the failure is sticky in the PJRT client: drop the backend
                # so the next attempt re-opens the devices
                try:
                    import jax

                    jax.clear_caches()
                    import jax.extend.backend

                    jax.extend.backend.clear_backends()
                except Exception:
                    pass
                time.sleep(15 * (attempt + 1))
                continue
            raise
    raise last


def kernel(x, weight_2bit, weight_scale, bias):
    res = run(make_in_maps(x, weight_2bit, weight_scale, bias))
    out = np.concatenate([r["out"] for r in res.results], axis=0)
    return np.ascontiguousarray(out.reshape(B, S, N))



# revision 2
# speedup vs baseline: 1.3186x; 1.3186x over previous
"""BitLinear 2-bit quantized linear layer on 8 TRN2 NeuronCores.

Math: reference computes
    a      = clip(max|x| over last dim, EPS)
    out    = ((x/a) @ W_deq^T) * (a*scale) + bias,  W_deq = QUANT_LEVELS[codes]
The per-row absmax normalization cancels exactly, so
    out == (x*scale) @ Wc^T + bias,  Wc = codes - 1.5.

Speed: the PE streams its moving operand at 2 bytes/cycle/partition, so fp8
matmuls in DoubleRow perf mode (2 fp8 lanes per cycle, contraction 256 per
instruction) run at exactly 2x the bf16 MAC rate (measured 216 ns per
[K=256]x[128,512] MM, same as a bf16 [K=128] MM). Pure-fp8 x would exceed the
2e-2 error budget (measured 2.5e-2), so K=4096 is split: 2048 k's go through
e4m3 DoubleRow (8 MMs/tile-pair) and 2048 k's through fp16 (16 MMs/pair,
quantization error negligible). 24 MMs/pair instead of 32 -> ~332us PE time.
The fp8 half is chosen as the 2048 k-columns with the smallest total e4m3
quantization error energy (host-side, shaves ~2% off the error).
Weights {+-0.5,+-1.5} are exact in e4m3 and fp16; weight_scale is folded into
x on the host before quantization.

Sharding: data-parallel over the 8192 = 4*2048 (batch*seq) rows; each of the
8 cores computes a [1024, 4096] slice of the output with the full weight.
"""

import time

import numpy as np
import ml_dtypes

import concourse.mybir as mybir
from concourse import bacc
from concourse.tile import TileContext
from concourse.bass_utils import run_bass_kernel_spmd

N_CORES = 8
B, S, D_IN, D_OUT = 4, 2048, 4096, 4096
M_TOTAL = B * S              # 8192 rows
M = M_TOTAL // N_CORES       # 1024 rows per core
K = D_IN
N = D_OUT
P = 128                      # partitions
NF = 512                     # psum free dim (one PSUM bank of fp32)
NI = N // NF                 # 8 n-chunks
MI = M // P                  # 8 m-tiles
T8 = 8                       # fp8 DoubleRow k-tiles (256 k each)
K8 = T8 * 256                # 2048 k's via fp8
TH = (K - K8) // P           # 16 fp16 k-tiles (128 k each)

BF16 = mybir.dt.bfloat16
F16 = mybir.dt.float16
F8 = mybir.dt.float8e4
F32 = mybir.dt.float32
DR = mybir.MatmulPerfMode.DoubleRow


def build():
    nc = bacc.Bacc()
    # x8: [p, t, i, m] = e4m3 x at k = sel[t*256 + i*128 + p]
    x8_d = nc.declare_dram_parameter("x8", [P, T8 * 2 * M], F8, isOutput=False)
    # xh: [p, kk, m] = fp16 x at k = rest[kk*128 + p]
    xh_d = nc.declare_dram_parameter("xh", [P, TH * M], F16, isOutput=False)
    # w8: [p, ni, t, i, col]
    w8_d = nc.declare_dram_parameter("w8", [P, NI * T8 * 2 * NF], F8, isOutput=False)
    # wh: [p, ni, kk, col]
    wh_d = nc.declare_dram_parameter("wh", [P, NI * TH * NF], F16, isOutput=False)
    bias_d = nc.declare_dram_parameter("bias", [P, N], F32, isOutput=False)
    out_d = nc.declare_dram_parameter("out", [M, N], F32, isOutput=True)

    x8_v = x8_d[:].rearrange("p (t i m) -> p t i m", t=T8, i=2)
    xh_v = xh_d[:].rearrange("p (kk m) -> p kk m", kk=TH)
    w8_v = w8_d[:].rearrange("p (ni t i c) -> p ni t i c", ni=NI, t=T8, i=2)
    wh_v = wh_d[:].rearrange("p (ni kk c) -> p ni kk c", ni=NI, kk=TH)

    with TileContext(nc) as tc:
        with (
            tc.tile_pool(name="xpool", bufs=1) as xpool,
            tc.tile_pool(name="bpool", bufs=1) as bpool,
            tc.tile_pool(name="w8pool", bufs=2) as w8pool,
            tc.tile_pool(name="whpool", bufs=2) as whpool,
            tc.tile_pool(name="opool", bufs=6) as opool,
            tc.tile_pool(name="ppool", bufs=8, space="PSUM") as ppool,
        ):
            x8t = xpool.tile([P, T8, 2, M], F8, name="x8t")
            xht = xpool.tile([P, TH, M], F16, name="xht")
            w8c = w8pool.tile([P, T8, 2, NF], F8, name="w8c")
            whc = whpool.tile([P, TH, NF], F16, name="whc")

            # Startup stream: interleave x (scalar/ACT ring) and the ni=0
            # weight chunk (sync/SP ring) in k-order pieces so the t-major
            # ni=0 matmul wave can ride right behind the DMA stream.
            for t in range(T8):
                nc.scalar.dma_start(out=x8t[:, t, :, :], in_=x8_v[:, t, :, :])
                nc.sync.dma_start(out=w8c[:, t, :, :], in_=w8_v[:, 0, t, :, :])
            for kk in range(TH):
                nc.scalar.dma_start(out=xht[:, kk, :], in_=xh_v[:, kk, :])
                nc.sync.dma_start(out=whc[:, kk, :], in_=wh_v[:, 0, kk, :])
            bias_sb = bpool.tile([P, N], F32, name="bias_sb")
            nc.scalar.dma_start(out=bias_sb[:], in_=bias_d[:])

            # PE warmup: dummy matmuls keep the PE busy while the first data
            # chunks stream in so the HAM clock-gate reaches 2.4 GHz.
            warm_l = bpool.tile([P, P], BF16, name="warm_l")
            warm_r = bpool.tile([P, NF], BF16, name="warm_r")
            nc.vector.memset(warm_l[:], 0.0)
            nc.vector.memset(warm_r[:], 0.0)

            def epilogue(ps, mi, nsl):
                ot = opool.tile([P, NF], F32, name="ot")
                nc.vector.tensor_add(out=ot[:], in0=ps[:], in1=bias_sb[:, nsl])
                nc.sync.dma_start(out=out_d[mi * P:(mi + 1) * P, nsl], in_=ot[:])

            def pair_mms(ps, w8cur, whcur, mi, csl=slice(0, NF)):
                msl = slice(mi * P, (mi + 1) * P)
                for t in range(T8):
                    nc.tensor.matmul(
                        ps[:], lhsT=x8t[:, t, :, msl], rhs=w8cur[:, t, :, csl],
                        start=(t == 0), stop=False, perf_mode=DR,
                    )
                for kk in range(TH):
                    nc.tensor.matmul(
                        ps[:], lhsT=xht[:, kk, msl], rhs=whcur[:, kk, csl],
                        start=False, stop=(kk == TH - 1),
                    )

            w8cur, whcur = w8c, whc
            for ni in range(NI):
                nsl = slice(ni * NF, (ni + 1) * NF)
                w8n = whn = None
                if ni + 1 < NI:
                    w8n = w8pool.tile([P, T8, 2, NF], F8, name="w8c")
                    whn = whpool.tile([P, TH, NF], F16, name="whc")
                if ni == 0:
                    # t-major over all 8 psum banks so the PE accumulates into
                    # every mi's bank as each k-slice of x/w arrives.
                    pss = [ppool.tile([P, NF], F32, name="ps") for _ in range(MI)]
                    for _ in range(12):
                        nc.tensor.matmul(
                            pss[MI - 1][:], lhsT=warm_l[:], rhs=warm_r[:],
                            start=True, stop=True,
                        )
                    for t in range(T8):
                        for mi in range(MI):
                            nc.tensor.matmul(
                                pss[mi][:],
                                lhsT=x8t[:, t, :, mi * P:(mi + 1) * P],
                                rhs=w8cur[:, t, :, :],
                                start=(t == 0), stop=False, perf_mode=DR,
                            )
                    for kk in range(TH):
                        for mi in range(MI):
                            nc.tensor.matmul(
                                pss[mi][:],
                                lhsT=xht[:, kk, mi * P:(mi + 1) * P],
                                rhs=whcur[:, kk, :],
                                start=False, stop=(kk == TH - 1),
                            )
                        # delay the ni=1 prefetch until the startup stream is
                        # done so they don't race for HBM bandwidth
                        if kk == TH - 1:
                            for t in range(T8):
                                nc.sync.dma_start(
                                    out=w8n[:, t, :, :], in_=w8_v[:, 1, t, :, :])
                            for k2 in range(0, TH, 4):
                                nc.sync.dma_start(
                                    out=whn[:, k2:k2 + 4, :],
                                    in_=wh_v[:, 1, k2:k2 + 4, :])
                    for mi in range(MI):
                        epilogue(pss[mi], mi, nsl)
                else:
                    for mi in range(MI):
                        last_pair = ni == NI - 1 and mi == MI - 1
                        if last_pair:
                            # split the final pair into two half-width chains
                            # so the kernel tail only drains a short epilogue
                            hf = NF // 2
                            for half in range(2):
                                ps = ppool.tile([P, hf], F32, name="ps")
                                pair_mms(ps, w8cur, whcur, mi,
                                         csl=slice(half * hf, (half + 1) * hf))
                                hsl = slice(ni * NF + half * hf,
                                            ni * NF + (half + 1) * hf)
                                ot = opool.tile([P, hf], F32, name="ot")
                                nc.vector.tensor_add(
                                    out=ot[:], in0=ps[:], in1=bias_sb[:, hsl])
                                nc.sync.dma_start(
                                    out=out_d[mi * P:(mi + 1) * P, hsl], in_=ot[:])
                            continue
                        ps = ppool.tile([P, NF], F32, name="ps")
                        pair_mms(ps, w8cur, whcur, mi)
                        epilogue(ps, mi, nsl)
                        # spread next-chunk weight DMA issue across the phase
                        if w8n is not None and mi < 4:
                            nc.sync.dma_start(
                                out=w8n[:, 2 * mi:2 * mi + 2, :, :],
                                in_=w8_v[:, ni + 1, 2 * mi:2 * mi + 2, :, :])
                        if whn is not None and 4 <= mi < 8:
                            k2 = (mi - 4) * 4
                            nc.sync.dma_start(
                                out=whn[:, k2:k2 + 4, :],
                                in_=wh_v[:, ni + 1, k2:k2 + 4, :])
                w8cur, whcur = w8n, whn
    nc.finalize()
    return nc


_NC = None


def _get_nc():
    global _NC
    if _NC is None:
        _NC = build()
    return _NC


def make_in_maps(x, weight_2bit, weight_scale, bias):
    x = np.asarray(x).reshape(M_TOTAL, K)
    codes = np.asarray(weight_2bit)
    ws = np.float32(np.asarray(weight_scale).reshape(-1)[0])
    b = np.asarray(bias).astype(np.float32)

    xs = x * ws if ws != np.float32(1.0) else x
    x8_full = xs.astype(ml_dtypes.float8_e4m3)
    # pick the 2048 k-columns with the least e4m3 quantization error energy
    # for the fp8 half; the rest go through fp16
    d2 = ((x8_full.astype(np.float32) - xs) ** 2).sum(axis=0)
    order = np.argsort(d2)
    sel = np.sort(order[:K8])
    rest = np.sort(order[K8:])

    Wc = codes.astype(np.float32) - np.float32(1.5)              # [N, K]
    w8 = np.ascontiguousarray(Wc[:, sel].T).astype(ml_dtypes.float8_e4m3)
    # [k8, n] -> [p, ni, t, i, col]
    w8 = w8.reshape(T8, 2, P, NI, NF).transpose(2, 3, 0, 1, 4)
    w8 = np.ascontiguousarray(w8.reshape(P, NI * T8 * 2 * NF))
    wh = np.ascontiguousarray(Wc[:, rest].T).astype(np.float16)
    wh = wh.reshape(TH, P, NI, NF).transpose(1, 2, 0, 3)
    wh = np.ascontiguousarray(wh.reshape(P, NI * TH * NF))

    bias_rep = np.ascontiguousarray(np.broadcast_to(b, (P, N)))

    in_maps = []
    for c in range(N_CORES):
        rows = slice(c * M, (c + 1) * M)
        x8c = np.ascontiguousarray(x8_full[rows][:, sel].T)       # [k8, m]
        x8c = x8c.reshape(T8, 2, P, M).transpose(2, 0, 1, 3)
        x8c = np.ascontiguousarray(x8c.reshape(P, T8 * 2 * M))
        xhc = np.ascontiguousarray(xs[rows][:, rest].T.astype(np.float16))
        xhc = xhc.reshape(TH, P, M).transpose(1, 0, 2)
        xhc = np.ascontiguousarray(xhc.reshape(P, TH * M))
        in_maps.append({"x8": x8c, "xh": xhc, "w8": w8, "wh": wh,
                        "bias": bias_rep})
    return in_maps


def run(in_maps, trace=False, **kw):
    # The axon-tunneled devices occasionally fail a fresh process's first
    # execution with NRT_EXEC_UNIT_UNRECOVERABLE; an identical retry succeeds.
    last = None
    for attempt in range(4):
        try:
            return run_bass_kernel_spmd(
                _get_nc(), in_maps, list(range(N_CORES)), trace=trace, **kw
            )
        except Exception as e:
            last = e
            msg = str(e)
            if "UNAVAILABLE" in msg or "unrecoverable" in msg.lower():
                # the failure is sticky in the PJRT client: drop the backend
                # so the next attempt re-opens the devices
                try:
                    import jax

                    jax.clear_caches()
                    import jax.extend.backend

                    jax.extend.backend.clear_backends()
                except Exception:
                    pass
                time.sleep(15 * (attempt + 1))
                continue
            raise
    raise last


def kernel(x, weight_2bit, weight_scale, bias):
    res = run(make_in_maps(x, weight_2bit, weight_scale, bias))
    out = np.concatenate([r["out"] for r in res.results], axis=0)
    return np.ascontiguousarray(out.reshape(B, S, N))
